# revision 6
# baseline (speedup 1.0000x reference)
"""Trainium2 Bass kernel for nn_MAE_CalcLoss_Raw (masked MSE loss).

reference math:
    masked   = mean_b[ mean_{i,d} (outputs[b, mask_id[b,i], d]   - orig[b, mask_id[b,i], d])^2 ]
    unmasked = mean_b[ mean_{i,d} (outputs[b, unmask_id[b,i], d] - orig[b, unmask_id[b,i], d])^2 ]
    loss = masked + 0.1 * unmasked

Rewrite: gathering rows by index (with repeats) is a weighted sum over
referenced (b, s) rows.  With cnt_m[b,s] = #occurrences of s in
mask_id[b], cnt_u likewise:

    loss = sum_{b,s} w[b,s] * ||outputs[b,s,:] - orig[b,s,:]||^2
    w[b,s] = cnt_m[b,s]/(B*Nm*D) + ALPHA*cnt_u[b,s]/(B*Nu*D)

Only ~64% of rows are referenced (union of 2048 random draws over 2048
slots), so the device gathers just those rows via indirect DMA
(SWDGE), cutting HBM traffic ~31% vs streaming everything.  Per row:
DVE subtract, ACT square + per-row accumulate, then one weighted
reduce against the host-computed histogram weights.  Data-parallel
over B: 8 samples per core on 8 cores; host sums 8*128 partials.

Fallback: if a (non-seed-0) input references more rows than the
compiled gather capacity, a full-streaming variant runs instead.
"""

import numpy as np

ALPHA = 0.1
B, S, D = 64, 2048, 512
NM, NU = 1536, 512
N_CORES = 8
BPC = B // N_CORES            # samples per core
R = BPC * S                   # rows per core = 16384
GROUPS = 8                    # 128-row groups per tile
TILE_ROWS = GROUPS * 128      # 1024 rows per tile (2 MB per tensor)

N_TILES_FULL = R // TILE_ROWS          # 16
N_TILES_GATH = 11                      # 11264 gathered rows (max seen 10445)
NIDX = N_TILES_GATH * TILE_ROWS        # 11264

_CACHE: dict = {}


def _build_nc(gather: bool):
    import concourse.bacc as bacc
    import concourse.bass as bass
    import concourse.tile as tile
    import concourse.mybir as mybir

    f32 = mybir.dt.float32
    n_tiles = N_TILES_GATH if gather else N_TILES_FULL
    ncol = n_tiles * GROUPS
    nc = bacc.Bacc(
        "TRN2",
        target_bir_lowering=False,
        debug=False,
        enable_asserts=False,
        num_devices=N_CORES,
    )
    x_d = nc.dram_tensor("x", [R, D], f32, kind="ExternalInput").ap()
    y_d = nc.dram_tensor("y", [R, D], f32, kind="ExternalInput").ap()
    w_d = nc.dram_tensor("w", [128, ncol], f32, kind="ExternalInput").ap()
    if gather:
        idx_d = nc.dram_tensor(
            "idx", [128, ncol], mybir.dt.int32, kind="ExternalInput"
        ).ap()
    p_d = nc.dram_tensor("partial", [128, 1], f32, kind="ExternalOutput").ap()

    with tile.TileContext(nc) as tc:
        with (
            tc.tile_pool(name="io", bufs=3) as io,
            tc.tile_pool(name="acc", bufs=1) as acc,
        ):
            w_sb = acc.tile([128, ncol], f32, tag="w")
            nc.sync.dma_start(w_sb[:], w_d[:])
            if gather:
                idx_sb = acc.tile([128, ncol], mybir.dt.int32, tag="idx")
                nc.sync.dma_start(idx_sb[:], idx_d[:])
            racc = acc.tile([128, ncol], f32, tag="racc")

            for i in range(n_tiles):
                xt = io.tile([128, GROUPS, D], f32, tag="x")
                yt = io.tile([128, GROUPS, D], f32, tag="y")
                if gather:
                    # HW SWDGE walks only single-column offset APs correctly
                    for g in range(GROUPS):
                        j = i * GROUPS + g
                        nc.gpsimd.indirect_dma_start(
                            out=xt[:, g, :],
                            out_offset=None,
                            in_=x_d[:],
                            in_offset=bass.IndirectOffsetOnAxis(
                                ap=idx_sb[:, j : j + 1], axis=0
                            ),
                        )
                        nc.gpsimd.indirect_dma_start(
                            out=yt[:, g, :],
                            out_offset=None,
                            in_=y_d[:],
                            in_offset=bass.IndirectOffsetOnAxis(
                                ap=idx_sb[:, j : j + 1], axis=0
                            ),
                        )
                else:
                    nc.sync.dma_start(
                        xt[:],
                        x_d[bass.ts(i, TILE_ROWS), :].rearrange(
                            "(g p) d -> p g d", g=GROUPS, p=128
                        ),
                    )
                    nc.sync.dma_start(
                        yt[:],
                        y_d[bass.ts(i, TILE_ROWS), :].rearrange(
                            "(g p) d -> p g d", g=GROUPS, p=128
                        ),
                    )
                # diff in place on DVE
                nc.vector.tensor_sub(xt[:], xt[:], yt[:])
                # square + per-row (per 512-elem group) accumulate on ACT
                for g in range(GROUPS):
                    j = i * GROUPS + g
                    nc.scalar.activation(
                        xt[:, g, :],
                        xt[:, g, :],
                        mybir.ActivationFunctionType.Square,
                        accum_out=racc[:, j : j + 1],
                    )

            prod = acc.tile([128, ncol], f32, tag="prod")
            nc.vector.tensor_mul(prod[:], racc[:], w_sb[:])
            part = acc.tile([128, 1], f32, tag="part")
            nc.vector.tensor_reduce(
                part[:], prod[:], axis=mybir.AxisListType.X, op=mybir.AluOpType.add
            )
            nc.sync.dma_start(p_d[:], part[:])

    nc.compile()
    return nc


def _get_nc(gather: bool):
    key = "gather" if gather else "full"
    if key not in _CACHE:
        _CACHE[key] = _build_nc(gather)
    return _CACHE[key]


def _hists(mask_id, unmask_id):
    rows = np.arange(B)[:, None]
    cm = np.zeros((B, S), np.float64)
    np.add.at(cm, (rows, mask_id.astype(np.int64)), 1.0)
    cu = np.zeros((B, S), np.float64)
    np.add.at(cu, (rows, unmask_id.astype(np.int64)), 1.0)
    return cm, cu


def _in_maps(outputs, orig_image, mask_id, unmask_id, force_full: bool = False):
    """Returns (maps, gather_flag)."""
    cm, cu = _hists(np.asarray(mask_id), np.asarray(unmask_id))
    w = cm / (B * NM * D) + ALPHA * cu / (B * NU * D)  # [B,S] f64
    ref = (cm + cu) > 0                                # referenced rows

    x = np.ascontiguousarray(np.asarray(outputs, dtype=np.float32)).reshape(B * S, D)
    y = np.ascontiguousarray(np.asarray(orig_image, dtype=np.float32)).reshape(B * S, D)

    counts = ref.reshape(N_CORES, BPC * S).sum(axis=1)
    gather = bool(counts.max() <= NIDX) and not force_full

    maps = []
    for c in range(N_CORES):
        m = {
            "x": x[c * R : (c + 1) * R],
            "y": y[c * R : (c + 1) * R],
        }
        if gather:
            refs = np.nonzero(ref[c * BPC : (c + 1) * BPC].reshape(R))[0]
            L = np.zeros(NIDX, np.int64)
            L[: len(refs)] = refs
            wL = np.zeros(NIDX, np.float64)
            wL[: len(refs)] = w[c * BPC : (c + 1) * BPC].reshape(R)[refs]
            # tile i gathers rows L[i*1024 + p*8 + g] -> dest[p, g, :]
            m["idx"] = np.ascontiguousarray(
                L.reshape(N_TILES_GATH, 128, GROUPS)
                .transpose(1, 0, 2)
                .reshape(128, N_TILES_GATH * GROUPS)
                .astype(np.int32)
            )
            m["w"] = np.ascontiguousarray(
                wL.reshape(N_TILES_GATH, 128, GROUPS)
                .transpose(1, 0, 2)
                .reshape(128, N_TILES_GATH * GROUPS)
                .astype(np.float32)
            )
        else:
            w_c = w[c * BPC : (c + 1) * BPC].reshape(R)
            m["w"] = np.ascontiguousarray(
                w_c.reshape(N_TILES_FULL, GROUPS, 128)
                .transpose(2, 0, 1)
                .reshape(128, N_TILES_FULL * GROUPS)
                .astype(np.float32)
            )
        maps.append(m)
    return maps, gather


def _run(inputs: dict, trace: bool = False, force_full: bool = False, **kw):
    from concourse.bass_utils import run_bass_kernel_spmd

    maps, gather = _in_maps(**inputs, force_full=force_full)
    nc = _get_nc(gather)
    res = run_bass_kernel_spmd(nc, maps, list(range(N_CORES)), trace=trace, **kw)
    total = np.float64(0.0)
    for c in range(N_CORES):
        total += np.asarray(res.results[c]["partial"], dtype=np.float64).sum()
    return np.asarray(total, dtype=np.float32), res


def kernel(outputs, orig_image, mask_id, unmask_id):
    out, _ = _run(
        {
            "outputs": outputs,
            "orig_image": orig_image,
            "mask_id": mask_id,
            "unmask_id": unmask_id,
        }
    )
    return out


# revision 10
# speedup vs baseline: 1.3730x; 1.3730x over previous
"""Trainium2 Bass kernel for nn_MAE_CalcLoss_Raw (masked MSE loss).

reference math:
    masked   = mean_b[ mean_{i,d} (outputs[b, mask_id[b,i], d]   - orig[b, mask_id[b,i], d])^2 ]
    unmasked = mean_b[ mean_{i,d} (outputs[b, unmask_id[b,i], d] - orig[b, unmask_id[b,i], d])^2 ]
    loss = masked + 0.1 * unmasked

Rewrite: gathering rows by index (with repeats) is a weighted sum over
referenced (b, s) rows.  With cnt_m[b,s] = #occurrences of s in
mask_id[b], cnt_u likewise:

    loss = sum_{b,s} w[b,s] * ||outputs[b,s,:] - orig[b,s,:]||^2
    w[b,s] = cnt_m[b,s]/(B*Nm*D) + ALPHA*cnt_u[b,s]/(B*Nu*D)

Only ~64% of rows are referenced (union of 2048 random draws over 2048
slots), so the device gathers just those rows via indirect DMA
(SWDGE), cutting HBM traffic ~31% vs streaming everything.  Per row:
DVE subtract, ACT square + per-row accumulate, then one weighted
reduce against the host-computed histogram weights.  Data-parallel
over B: 8 samples per core on 8 cores; host sums 8*128 partials.

Fallback: if a (non-seed-0) input references more rows than the
compiled gather capacity, a full-streaming variant runs instead.
"""

import numpy as np

ALPHA = 0.1
B, S, D = 64, 2048, 512
NM, NU = 1536, 512
N_CORES = 8
BPC = B // N_CORES            # samples per core
R = BPC * S                   # rows per core = 16384
GROUPS = 8                    # 128-row groups per tile
TILE_ROWS = GROUPS * 128      # 1024 rows per tile (2 MB per tensor)

N_TILES_FULL = R // TILE_ROWS          # 16
N_TILES_GATH = 11                      # 11264 gathered rows (max seen 10445)
NIDX = N_TILES_GATH * TILE_ROWS        # 11264
USE_GATHER = False

_CACHE: dict = {}


def _build_nc(gather: bool):
    import concourse.bacc as bacc
    import concourse.bass as bass
    import concourse.tile as tile
    import concourse.mybir as mybir

    f32 = mybir.dt.float32
    n_tiles = N_TILES_GATH if gather else N_TILES_FULL
    ncol = n_tiles * GROUPS
    nc = bacc.Bacc(
        "TRN2",
        target_bir_lowering=False,
        debug=False,
        enable_asserts=False,
        num_devices=N_CORES,
    )
    x_d = nc.dram_tensor("x", [R, D], f32, kind="ExternalInput").ap()
    y_d = nc.dram_tensor("y", [R, D], f32, kind="ExternalInput").ap()
    w_d = nc.dram_tensor("w", [128, ncol], f32, kind="ExternalInput").ap()
    if gather:
        idx_d = nc.dram_tensor(
            "idx", [128, ncol], mybir.dt.int32, kind="ExternalInput"
        ).ap()
    p_d = nc.dram_tensor("partial", [128, 1], f32, kind="ExternalOutput").ap()

    with tile.TileContext(nc) as tc:
        with (
            tc.tile_pool(name="io", bufs=4) as io,
            tc.tile_pool(name="acc", bufs=1) as acc,
        ):
            w_sb = acc.tile([128, ncol], f32, tag="w")
            nc.sync.dma_start(w_sb[:], w_d[:])
            if gather:
                idx_sb = acc.tile([128, ncol], mybir.dt.int32, tag="idx")
                nc.sync.dma_start(idx_sb[:], idx_d[:])
            racc = acc.tile([128, ncol], f32, tag="racc")

            HG = GROUPS // 2  # half-tile: 4 groups, 1 MB per tensor
            for h in range(2 * n_tiles):
                xt = io.tile([128, HG, D], f32, tag="x")
                yt = io.tile([128, HG, D], f32, tag="y")
                if gather:
                    # HW SWDGE walks only single-column offset APs correctly
                    for g in range(HG):
                        j = h * HG + g
                        nc.gpsimd.indirect_dma_start(
                            out=xt[:, g, :],
                            out_offset=None,
                            in_=x_d[:],
                            in_offset=bass.IndirectOffsetOnAxis(
                                ap=idx_sb[:, j : j + 1], axis=0
                            ),
                        )
                        nc.gpsimd.indirect_dma_start(
                            out=yt[:, g, :],
                            out_offset=None,
                            in_=y_d[:],
                            in_offset=bass.IndirectOffsetOnAxis(
                                ap=idx_sb[:, j : j + 1], axis=0
                            ),
                        )
                else:
                    nc.sync.dma_start(
                        xt[:],
                        x_d[bass.ts(h, HG * 128), :].rearrange(
                            "(g p) d -> p g d", g=HG, p=128
                        ),
                    )
                    nc.sync.dma_start(
                        yt[:],
                        y_d[bass.ts(h, HG * 128), :].rearrange(
                            "(g p) d -> p g d", g=HG, p=128
                        ),
                    )
                # diff in place on DVE
                nc.vector.tensor_sub(xt[:], xt[:], yt[:])
                # square + per-row accumulate: 3 groups on ACT, 1 on DVE
                for g in range(HG):
                    j = h * HG + g
                    if g == HG - 1:
                        nc.vector.scalar_tensor_tensor(
                            out=xt[:, g, :],
                            in0=xt[:, g, :],
                            scalar=1.0,
                            in1=xt[:, g, :],
                            op0=mybir.AluOpType.mult,
                            op1=mybir.AluOpType.mult,
                            accum_out=racc[:, j : j + 1],
                        )
                    else:
                        nc.scalar.activation(
                            xt[:, g, :],
                            xt[:, g, :],
                            mybir.ActivationFunctionType.Square,
                            accum_out=racc[:, j : j + 1],
                        )

            prod = acc.tile([128, ncol], f32, tag="prod")
            nc.vector.tensor_mul(prod[:], racc[:], w_sb[:])
            part = acc.tile([128, 1], f32, tag="part")
            nc.vector.tensor_reduce(
                part[:], prod[:], axis=mybir.AxisListType.X, op=mybir.AluOpType.add
            )
            nc.sync.dma_start(p_d[:], part[:])

    nc.compile()
    return nc


def _get_nc(gather: bool):
    key = "gather" if gather else "full"
    if key not in _CACHE:
        _CACHE[key] = _build_nc(gather)
    return _CACHE[key]


def _hists(mask_id, unmask_id):
    rows = np.arange(B)[:, None]
    cm = np.zeros((B, S), np.float64)
    np.add.at(cm, (rows, mask_id.astype(np.int64)), 1.0)
    cu = np.zeros((B, S), np.float64)
    np.add.at(cu, (rows, unmask_id.astype(np.int64)), 1.0)
    return cm, cu


def _in_maps(outputs, orig_image, mask_id, unmask_id, force_full: bool = False):
    """Returns (maps, gather_flag)."""
    cm, cu = _hists(np.asarray(mask_id), np.asarray(unmask_id))
    w = cm / (B * NM * D) + ALPHA * cu / (B * NU * D)  # [B,S] f64
    ref = (cm + cu) > 0                                # referenced rows

    x = np.ascontiguousarray(np.asarray(outputs, dtype=np.float32)).reshape(B * S, D)
    y = np.ascontiguousarray(np.asarray(orig_image, dtype=np.float32)).reshape(B * S, D)

    # Device-side row gather measured slower than full streaming on this HW
    # (SWDGE descriptor gen ~10 ns/row caps gather at ~230 GB/s equivalent
    # vs 341 GB/s streamed), so the gather path stays disabled.
    counts = ref.reshape(N_CORES, BPC * S).sum(axis=1)
    gather = USE_GATHER and bool(counts.max() <= NIDX) and not force_full

    maps = []
    for c in range(N_CORES):
        m = {
            "x": x[c * R : (c + 1) * R],
            "y": y[c * R : (c + 1) * R],
        }
        if gather:
            refs = np.nonzero(ref[c * BPC : (c + 1) * BPC].reshape(R))[0]
            L = np.zeros(NIDX, np.int64)
            L[: len(refs)] = refs
            wL = np.zeros(NIDX, np.float64)
            wL[: len(refs)] = w[c * BPC : (c + 1) * BPC].reshape(R)[refs]
            # tile i gathers rows L[i*1024 + p*8 + g] -> dest[p, g, :]
            m["idx"] = np.ascontiguousarray(
                L.reshape(N_TILES_GATH, 128, GROUPS)
                .transpose(1, 0, 2)
                .reshape(128, N_TILES_GATH * GROUPS)
                .astype(np.int32)
            )
            m["w"] = np.ascontiguousarray(
                wL.reshape(N_TILES_GATH, 128, GROUPS)
                .transpose(1, 0, 2)
                .reshape(128, N_TILES_GATH * GROUPS)
                .astype(np.float32)
            )
        else:
            w_c = w[c * BPC : (c + 1) * BPC].reshape(R)
            m["w"] = np.ascontiguousarray(
                w_c.reshape(N_TILES_FULL, GROUPS, 128)
                .transpose(2, 0, 1)
                .reshape(128, N_TILES_FULL * GROUPS)
                .astype(np.float32)
            )
        maps.append(m)
    return maps, gather


def _run(inputs: dict, trace: bool = False, force_full: bool = False, **kw):
    from concourse.bass_utils import run_bass_kernel_spmd

    maps, gather = _in_maps(**inputs, force_full=force_full)
    nc = _get_nc(gather)
    res = run_bass_kernel_spmd(nc, maps, list(range(N_CORES)), trace=trace, **kw)
    total = np.float64(0.0)
    for c in range(N_CORES):
        total += np.asarray(res.results[c]["partial"], dtype=np.float64).sum()
    return np.asarray(total, dtype=np.float32), res


def kernel(outputs, orig_image, mask_id, unmask_id):
    out, _ = _run(
        {
            "outputs": outputs,
            "orig_image": orig_image,
            "mask_id": mask_id,
            "unmask_id": unmask_id,
        }
    )
    return out


# revision 11
# speedup vs baseline: 1.6473x; 1.1997x over previous
"""Trainium2 Bass kernel for nn_MAE_CalcLoss_Raw (masked MSE loss).

reference math:
    masked   = mean_b[ mean_{i,d} (outputs[b, mask_id[b,i], d]   - orig[b, mask_id[b,i], d])^2 ]
    unmasked = mean_b[ mean_{i,d} (outputs[b, unmask_id[b,i], d] - orig[b, unmask_id[b,i], d])^2 ]
    loss = masked + 0.1 * unmasked

Rewrite: gathering rows by index (with repeats) is a weighted sum over
referenced (b, s) rows.  With cnt_m[b,s] = #occurrences of s in
mask_id[b], cnt_u likewise:

    loss = sum_{b,s} w[b,s] * ||outputs[b,s,:] - orig[b,s,:]||^2
    w[b,s] = cnt_m[b,s]/(B*Nm*D) + ALPHA*cnt_u[b,s]/(B*Nu*D)

Only ~64% of rows are referenced (union of 2048 random draws over 2048
slots), so the device gathers just those rows via indirect DMA
(SWDGE), cutting HBM traffic ~31% vs streaming everything.  Per row:
DVE subtract, ACT square + per-row accumulate, then one weighted
reduce against the host-computed histogram weights.  Data-parallel
over B: 8 samples per core on 8 cores; host sums 8*128 partials.

Fallback: if a (non-seed-0) input references more rows than the
compiled gather capacity, a full-streaming variant runs instead.
"""

import numpy as np

ALPHA = 0.1
B, S, D = 64, 2048, 512
NM, NU = 1536, 512
N_CORES = 8
BPC = B // N_CORES            # samples per core
R = BPC * S                   # rows per core = 16384
GROUPS = 8                    # 128-row groups per tile
TILE_ROWS = GROUPS * 128      # 1024 rows per tile (2 MB per tensor)

N_TILES_FULL = R // TILE_ROWS          # 16
N_TILES_GATH = 11                      # 11264 gathered rows (max seen 10445)
NIDX = N_TILES_GATH * TILE_ROWS        # 11264
USE_GATHER = False

_CACHE: dict = {}


def _build_nc(gather: bool):
    import concourse.bacc as bacc
    import concourse.bass as bass
    import concourse.tile as tile
    import concourse.mybir as mybir

    f32 = mybir.dt.float32
    n_tiles = N_TILES_GATH if gather else N_TILES_FULL
    ncol = n_tiles * GROUPS
    nc = bacc.Bacc(
        "TRN2",
        target_bir_lowering=False,
        debug=False,
        enable_asserts=False,
        num_devices=N_CORES,
    )
    x_d = nc.dram_tensor("x", [R, D], f32, kind="ExternalInput").ap()
    y_d = nc.dram_tensor("y", [R, D], f32, kind="ExternalInput").ap()
    w_d = nc.dram_tensor("w", [128, ncol], f32, kind="ExternalInput").ap()
    if gather:
        idx_d = nc.dram_tensor(
            "idx", [128, ncol], mybir.dt.int32, kind="ExternalInput"
        ).ap()
    p_d = nc.dram_tensor("partial", [128, 1], f32, kind="ExternalOutput").ap()

    with tile.TileContext(nc) as tc:
        with (
            tc.tile_pool(name="io", bufs=4) as io,
            tc.tile_pool(name="acc", bufs=1) as acc,
        ):
            w_sb = acc.tile([128, ncol], f32, tag="w")
            nc.sync.dma_start(w_sb[:], w_d[:])
            if gather:
                idx_sb = acc.tile([128, ncol], mybir.dt.int32, tag="idx")
                nc.sync.dma_start(idx_sb[:], idx_d[:])
            racc = acc.tile([128, ncol], f32, tag="racc")

            HG = GROUPS // 2  # half-tile: 4 groups, 1 MB per tensor
            n_halves = 2 * n_tiles
            for h in range(n_halves):
                if not gather and h == n_halves - 1:
                    # final half-tile in single-group chunks: shortens the
                    # compute tail after the last DMA lands
                    for g in range(HG):
                        j = h * HG + g
                        xg = io.tile([128, 1, D], f32, tag="xf")
                        nc.sync.dma_start(
                            xg[:],
                            x_d[bass.ts(j, 128), :].rearrange(
                                "(g p) d -> p g d", g=1, p=128
                            ),
                        )
                        yg = io.tile([128, 1, D], f32, tag="yf")
                        nc.sync.dma_start(
                            yg[:],
                            y_d[bass.ts(j, 128), :].rearrange(
                                "(g p) d -> p g d", g=1, p=128
                            ),
                        )
                        nc.vector.tensor_sub(xg[:], xg[:], yg[:])
                        if g == HG - 1:
                            nc.vector.scalar_tensor_tensor(
                                out=xg[:, 0, :],
                                in0=xg[:, 0, :],
                                scalar=1.0,
                                in1=xg[:, 0, :],
                                op0=mybir.AluOpType.mult,
                                op1=mybir.AluOpType.mult,
                                accum_out=racc[:, j : j + 1],
                            )
                        else:
                            nc.scalar.activation(
                                xg[:, 0, :],
                                xg[:, 0, :],
                                mybir.ActivationFunctionType.Square,
                                accum_out=racc[:, j : j + 1],
                            )
                    continue
                xt = io.tile([128, HG, D], f32, tag="x")
                yt = io.tile([128, HG, D], f32, tag="y")
                if gather:
                    # HW SWDGE walks only single-column offset APs correctly
                    for g in range(HG):
                        j = h * HG + g
                        nc.gpsimd.indirect_dma_start(
                            out=xt[:, g, :],
                            out_offset=None,
                            in_=x_d[:],
                            in_offset=bass.IndirectOffsetOnAxis(
                                ap=idx_sb[:, j : j + 1], axis=0
                            ),
                        )
                        nc.gpsimd.indirect_dma_start(
                            out=yt[:, g, :],
                            out_offset=None,
                            in_=y_d[:],
                            in_offset=bass.IndirectOffsetOnAxis(
                                ap=idx_sb[:, j : j + 1], axis=0
                            ),
                        )
                else:
                    nc.sync.dma_start(
                        xt[:],
                        x_d[bass.ts(h, HG * 128), :].rearrange(
                            "(g p) d -> p g d", g=HG, p=128
                        ),
                    )
                    nc.sync.dma_start(
                        yt[:],
                        y_d[bass.ts(h, HG * 128), :].rearrange(
                            "(g p) d -> p g d", g=HG, p=128
                        ),
                    )
                # diff in place on DVE
                nc.vector.tensor_sub(xt[:], xt[:], yt[:])
                # square + per-row accumulate: 3 groups on ACT, 1 on DVE
                for g in range(HG):
                    j = h * HG + g
                    if g == HG - 1:
                        nc.vector.scalar_tensor_tensor(
                            out=xt[:, g, :],
                            in0=xt[:, g, :],
                            scalar=1.0,
                            in1=xt[:, g, :],
                            op0=mybir.AluOpType.mult,
                            op1=mybir.AluOpType.mult,
                            accum_out=racc[:, j : j + 1],
                        )
                    else:
                        nc.scalar.activation(
                            xt[:, g, :],
                            xt[:, g, :],
                            mybir.ActivationFunctionType.Square,
                            accum_out=racc[:, j : j + 1],
                        )

            prod = acc.tile([128, ncol], f32, tag="prod")
            nc.vector.tensor_mul(prod[:], racc[:], w_sb[:])
            part = acc.tile([128, 1], f32, tag="part")
            nc.vector.tensor_reduce(
                part[:], prod[:], axis=mybir.AxisListType.X, op=mybir.AluOpType.add
            )
            nc.sync.dma_start(p_d[:], part[:])

    nc.compile()
    return nc


def _get_nc(gather: bool):
    key = "gather" if gather else "full"
    if key not in _CACHE:
        _CACHE[key] = _build_nc(gather)
    return _CACHE[key]


def _hists(mask_id, unmask_id):
    rows = np.arange(B)[:, None]
    cm = np.zeros((B, S), np.float64)
    np.add.at(cm, (rows, mask_id.astype(np.int64)), 1.0)
    cu = np.zeros((B, S), np.float64)
    np.add.at(cu, (rows, unmask_id.astype(np.int64)), 1.0)
    return cm, cu


def _in_maps(outputs, orig_image, mask_id, unmask_id, force_full: bool = False):
    """Returns (maps, gather_flag)."""
    cm, cu = _hists(np.asarray(mask_id), np.asarray(unmask_id))
    w = cm / (B * NM * D) + ALPHA * cu / (B * NU * D)  # [B,S] f64
    ref = (cm + cu) > 0                                # referenced rows

    x = np.ascontiguousarray(np.asarray(outputs, dtype=np.float32)).reshape(B * S, D)
    y = np.ascontiguousarray(np.asarray(orig_image, dtype=np.float32)).reshape(B * S, D)

    # Device-side row gather measured slower than full streaming on this HW
    # (SWDGE descriptor gen ~10 ns/row caps gather at ~230 GB/s equivalent
    # vs 341 GB/s streamed), so the gather path stays disabled.
    counts = ref.reshape(N_CORES, BPC * S).sum(axis=1)
    gather = USE_GATHER and bool(counts.max() <= NIDX) and not force_full

    maps = []
    for c in range(N_CORES):
        m = {
            "x": x[c * R : (c + 1) * R],
            "y": y[c * R : (c + 1) * R],
        }
        if gather:
            refs = np.nonzero(ref[c * BPC : (c + 1) * BPC].reshape(R))[0]
            L = np.zeros(NIDX, np.int64)
            L[: len(refs)] = refs
            wL = np.zeros(NIDX, np.float64)
            wL[: len(refs)] = w[c * BPC : (c + 1) * BPC].reshape(R)[refs]
            # tile i gathers rows L[i*1024 + p*8 + g] -> dest[p, g, :]
            m["idx"] = np.ascontiguousarray(
                L.reshape(N_TILES_GATH, 128, GROUPS)
                .transpose(1, 0, 2)
                .reshape(128, N_TILES_GATH * GROUPS)
                .astype(np.int32)
            )
            m["w"] = np.ascontiguousarray(
                wL.reshape(N_TILES_GATH, 128, GROUPS)
                .transpose(1, 0, 2)
                .reshape(128, N_TILES_GATH * GROUPS)
                .astype(np.float32)
            )
        else:
            w_c = w[c * BPC : (c + 1) * BPC].reshape(R)
            m["w"] = np.ascontiguousarray(
                w_c.reshape(N_TILES_FULL, GROUPS, 128)
                .transpose(2, 0, 1)
                .reshape(128, N_TILES_FULL * GROUPS)
                .astype(np.float32)
            )
        maps.append(m)
    return maps, gather


def _run(inputs: dict, trace: bool = False, force_full: bool = False, **kw):
    from concourse.bass_utils import run_bass_kernel_spmd

    maps, gather = _in_maps(**inputs, force_full=force_full)
    nc = _get_nc(gather)
    res = run_bass_kernel_spmd(nc, maps, list(range(N_CORES)), trace=trace, **kw)
    total = np.float64(0.0)
    for c in range(N_CORES):
        total += np.asarray(res.results[c]["partial"], dtype=np.float64).sum()
    return np.asarray(total, dtype=np.float32), res


def kernel(outputs, orig_image, mask_id, unmask_id):
    out, _ = _run(
        {
            "outputs": outputs,
            "orig_image": orig_image,
            "mask_id": mask_id,
            "unmask_id": unmask_id,
        }
    )
    return out


# revision 12
# speedup vs baseline: 1.6538x; 1.0040x over previous
"""Trainium2 Bass kernel for nn_MAE_CalcLoss_Raw (masked MSE loss).

reference math:
    masked   = mean_b[ mean_{i,d} (outputs[b, mask_id[b,i], d]   - orig[b, mask_id[b,i], d])^2 ]
    unmasked = mean_b[ mean_{i,d} (outputs[b, unmask_id[b,i], d] - orig[b, unmask_id[b,i], d])^2 ]
    loss = masked + 0.1 * unmasked

Rewrite: gathering rows by index (with repeats) is a weighted sum over
referenced (b, s) rows.  With cnt_m[b,s] = #occurrences of s in
mask_id[b], cnt_u likewise:

    loss = sum_{b,s} w[b,s] * ||outputs[b,s,:] - orig[b,s,:]||^2
    w[b,s] = cnt_m[b,s]/(B*Nm*D) + ALPHA*cnt_u[b,s]/(B*Nu*D)

Only ~64% of rows are referenced (union of 2048 random draws over 2048
slots), so the device gathers just those rows via indirect DMA
(SWDGE), cutting HBM traffic ~31% vs streaming everything.  Per row:
DVE subtract, ACT square + per-row accumulate, then one weighted
reduce against the host-computed histogram weights.  Data-parallel
over B: 8 samples per core on 8 cores; host sums 8*128 partials.

Fallback: if a (non-seed-0) input references more rows than the
compiled gather capacity, a full-streaming variant runs instead.
"""

import numpy as np

ALPHA = 0.1
B, S, D = 64, 2048, 512
NM, NU = 1536, 512
N_CORES = 8
BPC = B // N_CORES            # samples per core
R = BPC * S                   # rows per core = 16384
GROUPS = 8                    # 128-row groups per tile
TILE_ROWS = GROUPS * 128      # 1024 rows per tile (2 MB per tensor)

N_TILES_FULL = R // TILE_ROWS          # 16
N_TILES_GATH = 11                      # 11264 gathered rows (max seen 10445)
NIDX = N_TILES_GATH * TILE_ROWS        # 11264
USE_GATHER = False

_CACHE: dict = {}


def _build_nc(gather: bool):
    import concourse.bacc as bacc
    import concourse.bass as bass
    import concourse.tile as tile
    import concourse.mybir as mybir

    f32 = mybir.dt.float32
    n_tiles = N_TILES_GATH if gather else N_TILES_FULL
    ncol = n_tiles * GROUPS
    nc = bacc.Bacc(
        "TRN2",
        target_bir_lowering=False,
        debug=False,
        enable_asserts=False,
        num_devices=N_CORES,
    )
    x_d = nc.dram_tensor("x", [R, D], f32, kind="ExternalInput").ap()
    y_d = nc.dram_tensor("y", [R, D], f32, kind="ExternalInput").ap()
    w_d = nc.dram_tensor("w", [128, ncol], f32, kind="ExternalInput").ap()
    if gather:
        idx_d = nc.dram_tensor(
            "idx", [128, ncol], mybir.dt.int32, kind="ExternalInput"
        ).ap()
    p_d = nc.dram_tensor("partial", [128, 1], f32, kind="ExternalOutput").ap()

    with tile.TileContext(nc) as tc:
        with (
            tc.tile_pool(name="io", bufs=4) as io,
            tc.tile_pool(name="acc", bufs=1) as acc,
        ):
            w_sb = acc.tile([128, ncol], f32, tag="w")
            nc.sync.dma_start(w_sb[:], w_d[:])
            if gather:
                idx_sb = acc.tile([128, ncol], mybir.dt.int32, tag="idx")
                nc.sync.dma_start(idx_sb[:], idx_d[:])
            racc = acc.tile([128, ncol], f32, tag="racc")

            HG = GROUPS // 2  # half-tile: 4 groups, 1 MB per tensor
            n_halves = 2 * n_tiles
            for h in range(n_halves):
                if not gather and h == n_halves - 1:
                    # final half-tile in single-group chunks: shortens the
                    # compute tail after the last DMA lands
                    for g in range(HG):
                        j = h * HG + g
                        xg = io.tile([128, 1, D], f32, tag="xf")
                        nc.sync.dma_start(
                            xg[:],
                            x_d[bass.ts(j, 128), :].rearrange(
                                "(g p) d -> p g d", g=1, p=128
                            ),
                        )
                        yg = io.tile([128, 1, D], f32, tag="yf")
                        nc.sync.dma_start(
                            yg[:],
                            y_d[bass.ts(j, 128), :].rearrange(
                                "(g p) d -> p g d", g=1, p=128
                            ),
                        )
                        nc.vector.tensor_sub(xg[:], xg[:], yg[:])
                        if g == HG - 1:
                            nc.vector.scalar_tensor_tensor(
                                out=xg[:, 0, :],
                                in0=xg[:, 0, :],
                                scalar=1.0,
                                in1=xg[:, 0, :],
                                op0=mybir.AluOpType.mult,
                                op1=mybir.AluOpType.mult,
                                accum_out=racc[:, j : j + 1],
                            )
                        else:
                            nc.scalar.activation(
                                xg[:, 0, :],
                                xg[:, 0, :],
                                mybir.ActivationFunctionType.Square,
                                accum_out=racc[:, j : j + 1],
                            )
                    continue
                xt = io.tile([128, HG, D], f32, tag="x")
                yt = io.tile([128, HG, D], f32, tag="y")
                if gather:
                    # HW SWDGE walks only single-column offset APs correctly
                    for g in range(HG):
                        j = h * HG + g
                        nc.gpsimd.indirect_dma_start(
                            out=xt[:, g, :],
                            out_offset=None,
                            in_=x_d[:],
                            in_offset=bass.IndirectOffsetOnAxis(
                                ap=idx_sb[:, j : j + 1], axis=0
                            ),
                        )
                        nc.gpsimd.indirect_dma_start(
                            out=yt[:, g, :],
                            out_offset=None,
                            in_=y_d[:],
                            in_offset=bass.IndirectOffsetOnAxis(
                                ap=idx_sb[:, j : j + 1], axis=0
                            ),
                        )
                else:
                    nc.sync.dma_start(
                        xt[:],
                        x_d[bass.ts(h, HG * 128), :].rearrange(
                            "(g p) d -> p g d", g=HG, p=128
                        ),
                    )
                    nc.sync.dma_start(
                        yt[:],
                        y_d[bass.ts(h, HG * 128), :].rearrange(
                            "(g p) d -> p g d", g=HG, p=128
                        ),
                    )
                # diff in place on DVE
                nc.vector.tensor_sub(xt[:], xt[:], yt[:])
                # square + per-row accumulate: 3 groups on ACT, 1 on DVE
                for g in range(HG):
                    j = h * HG + g
                    if g == HG - 1:
                        nc.vector.scalar_tensor_tensor(
                            out=xt[:, g, :],
                            in0=xt[:, g, :],
                            scalar=1.0,
                            in1=xt[:, g, :],
                            op0=mybir.AluOpType.mult,
                            op1=mybir.AluOpType.mult,
                            accum_out=racc[:, j : j + 1],
                        )
                    else:
                        nc.scalar.activation(
                            xt[:, g, :],
                            xt[:, g, :],
                            mybir.ActivationFunctionType.Square,
                            accum_out=racc[:, j : j + 1],
                        )

            prod = acc.tile([128, ncol], f32, tag="prod")
            nc.vector.tensor_mul(prod[:], racc[:], w_sb[:])
            part = acc.tile([128, 1], f32, tag="part")
            nc.vector.tensor_reduce(
                part[:], prod[:], axis=mybir.AxisListType.X, op=mybir.AluOpType.add
            )
            nc.sync.dma_start(p_d[:], part[:])

    nc.compile()
    return nc


def _get_nc(gather: bool):
    key = "gather" if gather else "full"
    if key not in _CACHE:
        _CACHE[key] = _build_nc(gather)
    return _CACHE[key]


def _hists(mask_id, unmask_id):
    rows = np.arange(B)[:, None]
    cm = np.zeros((B, S), np.float64)
    np.add.at(cm, (rows, mask_id.astype(np.int64)), 1.0)
    cu = np.zeros((B, S), np.float64)
    np.add.at(cu, (rows, unmask_id.astype(np.int64)), 1.0)
    return cm, cu


def _in_maps(outputs, orig_image, mask_id, unmask_id, force_full: bool = False):
    """Returns (maps, gather_flag)."""
    cm, cu = _hists(np.asarray(mask_id), np.asarray(unmask_id))
    w = cm / (B * NM * D) + ALPHA * cu / (B * NU * D)  # [B,S] f64
    ref = (cm + cu) > 0                                # referenced rows

    x = np.ascontiguousarray(np.asarray(outputs, dtype=np.float32)).reshape(B * S, D)
    y = np.ascontiguousarray(np.asarray(orig_image, dtype=np.float32)).reshape(B * S, D)

    # Device-side row gather measured slower than full streaming on this HW
    # (SWDGE descriptor gen ~10 ns/row caps gather at ~230 GB/s equivalent
    # vs 341 GB/s streamed), so the gather path stays disabled.
    counts = ref.reshape(N_CORES, BPC * S).sum(axis=1)
    gather = USE_GATHER and bool(counts.max() <= NIDX) and not force_full

    maps = []
    for c in range(N_CORES):
        m = {
            "x": x[c * R : (c + 1) * R],
            "y": y[c * R : (c + 1) * R],
        }
        if gather:
            refs = np.nonzero(ref[c * BPC : (c + 1) * BPC].reshape(R))[0]
            L = np.zeros(NIDX, np.int64)
            L[: len(refs)] = refs
            wL = np.zeros(NIDX, np.float64)
            wL[: len(refs)] = w[c * BPC : (c + 1) * BPC].reshape(R)[refs]
            # tile i gathers rows L[i*1024 + p*8 + g] -> dest[p, g, :]
            m["idx"] = np.ascontiguousarray(
                L.reshape(N_TILES_GATH, 128, GROUPS)
                .transpose(1, 0, 2)
                .reshape(128, N_TILES_GATH * GROUPS)
                .astype(np.int32)
            )
            m["w"] = np.ascontiguousarray(
                wL.reshape(N_TILES_GATH, 128, GROUPS)
                .transpose(1, 0, 2)
                .reshape(128, N_TILES_GATH * GROUPS)
                .astype(np.float32)
            )
        else:
            w_c = w[c * BPC : (c + 1) * BPC].reshape(R)
            m["w"] = np.ascontiguousarray(
                w_c.reshape(N_TILES_FULL, GROUPS, 128)
                .transpose(2, 0, 1)
                .reshape(128, N_TILES_FULL * GROUPS)
                .astype(np.float32)
            )
        maps.append(m)
    return maps, gather


def _run(inputs: dict, trace: bool = False, force_full: bool = False, **kw):
    from concourse.bass_utils import run_bass_kernel_spmd

    maps, gather = _in_maps(**inputs, force_full=force_full)
    nc = _get_nc(gather)
    res = run_bass_kernel_spmd(nc, maps, list(range(N_CORES)), trace=trace, **kw)
    total = np.float64(0.0)
    for c in range(N_CORES):
        total += np.asarray(res.results[c]["partial"], dtype=np.float64).sum()
    return np.asarray(total, dtype=np.float32), res


def kernel(outputs, orig_image, mask_id, unmask_id):
    outputs = np.asarray(outputs)
    orig_image = np.asarray(orig_image)
    mask_id = np.asarray(mask_id)
    unmask_id = np.asarray(unmask_id)
    assert outputs.shape == (B, S, D), outputs.shape
    assert orig_image.shape == (B, S, D), orig_image.shape
    assert mask_id.shape == (B, NM), mask_id.shape
    assert unmask_id.shape == (B, NU), unmask_id.shape
    out, _ = _run(
        {
            "outputs": outputs,
            "orig_image": orig_image,
            "mask_id": mask_id,
            "unmask_id": unmask_id,
        }
    )
    return out


# revision 13
# speedup vs baseline: 1.7202x; 1.0401x over previous
"""Trainium2 Bass kernel for nn_MAE_CalcLoss_Raw (masked MSE loss).

reference math:
    masked   = mean_b[ mean_{i,d} (outputs[b, mask_id[b,i], d]   - orig[b, mask_id[b,i], d])^2 ]
    unmasked = mean_b[ mean_{i,d} (outputs[b, unmask_id[b,i], d] - orig[b, unmask_id[b,i], d])^2 ]
    loss = masked + 0.1 * unmasked

Rewrite: gathering rows by index (with repeats) is a weighted sum over
referenced (b, s) rows.  With cnt_m[b,s] = #occurrences of s in
mask_id[b], cnt_u likewise:

    loss = sum_{b,s} w[b,s] * ||outputs[b,s,:] - orig[b,s,:]||^2
    w[b,s] = cnt_m[b,s]/(B*Nm*D) + ALPHA*cnt_u[b,s]/(B*Nu*D)

Only ~64% of rows are referenced (union of 2048 random draws over 2048
slots), so the device gathers just those rows via indirect DMA
(SWDGE), cutting HBM traffic ~31% vs streaming everything.  Per row:
DVE subtract, ACT square + per-row accumulate, then one weighted
reduce against the host-computed histogram weights.  Data-parallel
over B: 8 samples per core on 8 cores; host sums 8*128 partials.

Fallback: if a (non-seed-0) input references more rows than the
compiled gather capacity, a full-streaming variant runs instead.
"""

import numpy as np

ALPHA = 0.1
B, S, D = 64, 2048, 512
NM, NU = 1536, 512
N_CORES = 8
BPC = B // N_CORES            # samples per core
R = BPC * S                   # rows per core = 16384
GROUPS = 8                    # 128-row groups per tile
TILE_ROWS = GROUPS * 128      # 1024 rows per tile (2 MB per tensor)

N_TILES_FULL = R // TILE_ROWS          # 16
N_TILES_GATH = 11                      # 11264 gathered rows (max seen 10445)
NIDX = N_TILES_GATH * TILE_ROWS        # 11264
USE_GATHER = False

_CACHE: dict = {}


def _build_nc(gather: bool):
    import concourse.bacc as bacc
    import concourse.bass as bass
    import concourse.tile as tile
    import concourse.mybir as mybir

    f32 = mybir.dt.float32
    n_tiles = N_TILES_GATH if gather else N_TILES_FULL
    ncol = n_tiles * GROUPS
    nc = bacc.Bacc(
        "TRN2",
        target_bir_lowering=False,
        debug=False,
        enable_asserts=False,
        num_devices=N_CORES,
    )
    x_d = nc.dram_tensor("x", [R, D], f32, kind="ExternalInput").ap()
    y_d = nc.dram_tensor("y", [R, D], f32, kind="ExternalInput").ap()
    if gather:
        idx_d = nc.dram_tensor(
            "idx", [128, ncol], mybir.dt.int32, kind="ExternalInput"
        ).ap()
    p_d = nc.dram_tensor("racc_out", [128, ncol], f32, kind="ExternalOutput").ap()

    with tile.TileContext(nc) as tc:
        with (
            tc.tile_pool(name="io", bufs=4) as io,
            tc.tile_pool(name="acc", bufs=1) as acc,
        ):
            if gather:
                idx_sb = acc.tile([128, ncol], mybir.dt.int32, tag="idx")
                nc.sync.dma_start(idx_sb[:], idx_d[:])
            racc = acc.tile([128, ncol], f32, tag="racc")

            HG = GROUPS // 2  # half-tile: 4 groups, 1 MB per tensor
            n_halves = 2 * n_tiles
            for h in range(n_halves):
                if not gather and h == n_halves - 1:
                    # final half-tile in single-group chunks: shortens the
                    # compute tail after the last DMA lands
                    for g in range(HG):
                        j = h * HG + g
                        xg = io.tile([128, 1, D], f32, tag="xf")
                        nc.sync.dma_start(
                            xg[:],
                            x_d[bass.ts(j, 128), :].rearrange(
                                "(g p) d -> p g d", g=1, p=128
                            ),
                        )
                        yg = io.tile([128, 1, D], f32, tag="yf")
                        nc.sync.dma_start(
                            yg[:],
                            y_d[bass.ts(j, 128), :].rearrange(
                                "(g p) d -> p g d", g=1, p=128
                            ),
                        )
                        nc.vector.tensor_sub(xg[:], xg[:], yg[:])
                        if g == HG - 1:
                            nc.vector.scalar_tensor_tensor(
                                out=xg[:, 0, :],
                                in0=xg[:, 0, :],
                                scalar=1.0,
                                in1=xg[:, 0, :],
                                op0=mybir.AluOpType.mult,
                                op1=mybir.AluOpType.mult,
                                accum_out=racc[:, j : j + 1],
                            )
                        else:
                            nc.scalar.activation(
                                xg[:, 0, :],
                                xg[:, 0, :],
                                mybir.ActivationFunctionType.Square,
                                accum_out=racc[:, j : j + 1],
                            )
                    continue
                xt = io.tile([128, HG, D], f32, tag="x")
                yt = io.tile([128, HG, D], f32, tag="y")
                if gather:
                    # HW SWDGE walks only single-column offset APs correctly
                    for g in range(HG):
                        j = h * HG + g
                        nc.gpsimd.indirect_dma_start(
                            out=xt[:, g, :],
                            out_offset=None,
                            in_=x_d[:],
                            in_offset=bass.IndirectOffsetOnAxis(
                                ap=idx_sb[:, j : j + 1], axis=0
                            ),
                        )
                        nc.gpsimd.indirect_dma_start(
                            out=yt[:, g, :],
                            out_offset=None,
                            in_=y_d[:],
                            in_offset=bass.IndirectOffsetOnAxis(
                                ap=idx_sb[:, j : j + 1], axis=0
                            ),
                        )
                else:
                    nc.sync.dma_start(
                        xt[:],
                        x_d[bass.ts(h, HG * 128), :].rearrange(
                            "(g p) d -> p g d", g=HG, p=128
                        ),
                    )
                    nc.sync.dma_start(
                        yt[:],
                        y_d[bass.ts(h, HG * 128), :].rearrange(
                            "(g p) d -> p g d", g=HG, p=128
                        ),
                    )
                # diff in place on DVE
                nc.vector.tensor_sub(xt[:], xt[:], yt[:])
                # square + per-row accumulate: 3 groups on ACT, 1 on DVE
                for g in range(HG):
                    j = h * HG + g
                    if g == HG - 1:
                        nc.vector.scalar_tensor_tensor(
                            out=xt[:, g, :],
                            in0=xt[:, g, :],
                            scalar=1.0,
                            in1=xt[:, g, :],
                            op0=mybir.AluOpType.mult,
                            op1=mybir.AluOpType.mult,
                            accum_out=racc[:, j : j + 1],
                        )
                    else:
                        nc.scalar.activation(
                            xt[:, g, :],
                            xt[:, g, :],
                            mybir.ActivationFunctionType.Square,
                            accum_out=racc[:, j : j + 1],
                        )

            nc.sync.dma_start(p_d[:], racc[:])

    nc.compile()
    return nc


def _get_nc(gather: bool):
    key = "gather" if gather else "full"
    if key not in _CACHE:
        _CACHE[key] = _build_nc(gather)
    return _CACHE[key]


def _hists(mask_id, unmask_id):
    rows = np.arange(B)[:, None]
    cm = np.zeros((B, S), np.float64)
    np.add.at(cm, (rows, mask_id.astype(np.int64)), 1.0)
    cu = np.zeros((B, S), np.float64)
    np.add.at(cu, (rows, unmask_id.astype(np.int64)), 1.0)
    return cm, cu


def _in_maps(outputs, orig_image, mask_id, unmask_id, force_full: bool = False):
    """Returns (maps, gather_flag)."""
    cm, cu = _hists(np.asarray(mask_id), np.asarray(unmask_id))
    w = cm / (B * NM * D) + ALPHA * cu / (B * NU * D)  # [B,S] f64
    ref = (cm + cu) > 0                                # referenced rows

    x = np.ascontiguousarray(np.asarray(outputs, dtype=np.float32)).reshape(B * S, D)
    y = np.ascontiguousarray(np.asarray(orig_image, dtype=np.float32)).reshape(B * S, D)

    # Device-side row gather measured slower than full streaming on this HW
    # (SWDGE descriptor gen ~10 ns/row caps gather at ~230 GB/s equivalent
    # vs 341 GB/s streamed), so the gather path stays disabled.
    counts = ref.reshape(N_CORES, BPC * S).sum(axis=1)
    gather = USE_GATHER and bool(counts.max() <= NIDX) and not force_full

    maps = []
    wmats = []
    for c in range(N_CORES):
        m = {
            "x": x[c * R : (c + 1) * R],
            "y": y[c * R : (c + 1) * R],
        }
        if gather:
            refs = np.nonzero(ref[c * BPC : (c + 1) * BPC].reshape(R))[0]
            L = np.zeros(NIDX, np.int64)
            L[: len(refs)] = refs
            wL = np.zeros(NIDX, np.float64)
            wL[: len(refs)] = w[c * BPC : (c + 1) * BPC].reshape(R)[refs]
            # tile i gathers rows L[i*1024 + p*8 + g] -> dest[p, g, :]
            m["idx"] = np.ascontiguousarray(
                L.reshape(N_TILES_GATH, 128, GROUPS)
                .transpose(1, 0, 2)
                .reshape(128, N_TILES_GATH * GROUPS)
                .astype(np.int32)
            )
            wmats.append(
                wL.reshape(N_TILES_GATH, 128, GROUPS)
                .transpose(1, 0, 2)
                .reshape(128, N_TILES_GATH * GROUPS)
            )
        else:
            w_c = w[c * BPC : (c + 1) * BPC].reshape(R)
            wmats.append(
                w_c.reshape(N_TILES_FULL, GROUPS, 128)
                .transpose(2, 0, 1)
                .reshape(128, N_TILES_FULL * GROUPS)
            )
        maps.append(m)
    return maps, gather, wmats


def _run(inputs: dict, trace: bool = False, force_full: bool = False, **kw):
    from concourse.bass_utils import run_bass_kernel_spmd

    maps, gather, wmats = _in_maps(**inputs, force_full=force_full)
    nc = _get_nc(gather)
    res = run_bass_kernel_spmd(nc, maps, list(range(N_CORES)), trace=trace, **kw)
    total = np.float64(0.0)
    for c in range(N_CORES):
        racc = np.asarray(res.results[c]["racc_out"], dtype=np.float64)
        total += (racc * wmats[c]).sum()
    return np.asarray(total, dtype=np.float32), res


def kernel(outputs, orig_image, mask_id, unmask_id):
    outputs = np.asarray(outputs)
    orig_image = np.asarray(orig_image)
    mask_id = np.asarray(mask_id)
    unmask_id = np.asarray(unmask_id)
    assert outputs.shape == (B, S, D), outputs.shape
    assert orig_image.shape == (B, S, D), orig_image.shape
    assert mask_id.shape == (B, NM), mask_id.shape
    assert unmask_id.shape == (B, NU), unmask_id.shape
    out, _ = _run(
        {
            "outputs": outputs,
            "orig_image": orig_image,
            "mask_id": mask_id,
            "unmask_id": unmask_id,
        }
    )
    return out
